# revision 67
# baseline (speedup 1.0000x reference)
"""CrossAttention kernel for 8 Trainium2 NeuronCores — v2 (Act-bound).

Reference (per batch element b, one core each):
    q = ts[b] @ q_w.T + q_b; k/v = llm[b] @ {k,v}_w.T + b
    per head h: ctx_h = softmax(q_h k_h^T / 8) v_h;  out = ctx @ o_w.T + o_b

v2 rationale: under the TimelineSim cost model the Act engine's exp
stream is the hard floor (16.8M softmax elements / 128 partitions x
0.83 ns = ~133 us).  v1 was PE-bound at ~197 us; v2 moves the big
projections to fp8e4 DoubleRow (0.5 cyc/row, half the passes) with
residual (hi+lo) splits to keep fp8 quantization error in check:

  QT/KT/V' schemes (contraction 1024 = 4 DR pairs of 256):
    qproj  fp8s: (ts_hi + ts_lo) x qw_hi          8 DR mm / tile
    kproj  fp8s: (llm_hi + llm_lo) x kw_hi        8 DR mm / (jt,sc)
    vproj  fp8s: (llm_hi + llm_lo) x vw_hi        8 DR mm / (st,jc)
  Weights are host-scaled x16 before the fp8 cast (their U(-1/32,1/32)
  range would land in fp8e4m3 denormals); the evacuation fuses the /16
  with the bias add (two-op tensor_scalar).  qt/kt are stored x8 in fp8
  (cuts the cast's denormal tail); the exp scale absorbs the /64.
  Scores stay fp8-DR; ctx + O-proj stay bf16.  Measured end-to-end rel
  err: 1.73e-2 (gate 2e-2).

No repacks: the host permutes the q/k weight COLUMNS so each projection
block (J,i) writes its output with partitions already in DoubleRow
order — qt_dr/kt_dr come straight from the bias-add evacuation, and the
scores matmuls slice them at row-group bases 0/32/64/96 (explicit
tile_position).  J indexes head-pair PAIRS: one kt_dr[J] serves pairs
2J and 2J+1.  kt J0 is "lite" (llm_hi only — heads 0/1 single-sided,
negligible in quadrature) so the first exp only waits on 2.5 MB of DMA.

Schedule: Act streams 2 exps/stage (2076 ns) for 64 stages; PE supplies
scores just-in-time and a slot table places every projection group
2+ stages before first use.  ctx lags ONE PAIR (8 stages) so V' spreads
at ~1 tile/stage; V's jc=1 half (heads 8-15) is not read until pair 4
and fills pairs 2-4.  The psc PSUM pair is reused every pair
(normalize(p) frees it before ctx(p+1) starts).  O-partials (d0..5)
run in pair 7; the tail does ctx(7,*), normalize, transposes(6,7) and
the d6/d7+partial-accumulate matmuls via identity matmul.

Input DMAs: ~15 large consolidated transfers, all on the sync queue
(the shared HWDGE device costs ~650 ns per issue regardless of size,
and DMAs on the Act queue would delay the exps in SEQ program order),
ordered so the startup-critical 2.5 MB lands first.
"""
import numpy as np
import ml_dtypes

D = 1024          # d_model
P = 512           # ts sequence length
S = 2048          # llm sequence length
H = 16            # heads
DH = 64           # head dim
NCORES = 8
NDT = D // 128    # 8 d-tiles
NDP = 4           # 4 d-pairs (DoubleRow: 256 contraction each)
NST = S // 128    # 16 s-tiles
NPT = P // 128    # 4 p-tiles
NPAIR = H // 2    # 8 head pairs

_BF16 = ml_dtypes.bfloat16
_F8 = ml_dtypes.float8_e4m3fn

_cached_nc = None


def _build_nc():
    import concourse.tile as tile
    from concourse import bacc, mybir

    f32 = mybir.dt.float32
    bf16 = mybir.dt.bfloat16

    nc = bacc.Bacc("TRN2", target_bir_lowering=False, debug=False,
                   num_devices=NCORES)

    f8 = mybir.dt.float8e4
    tsh = nc.declare_dram_parameter("tsh", [D, P], f8, isOutput=False)
    tsl = nc.declare_dram_parameter("tsl", [D, P], f8, isOutput=False)
    llmh = nc.declare_dram_parameter("llmh", [D, S], f8, isOutput=False)
    llml = nc.declare_dram_parameter("llml", [D, S], f8, isOutput=False)
    # combined q/k weights, columns [qj01|kj01|qrest|krest] (permuted)
    qkw = nc.declare_dram_parameter("qkw", [D, 2 * D], f8, isOutput=False)
    vwh = nc.declare_dram_parameter("vwh", [D, D], f8, isOutput=False)
    owT = nc.declare_dram_parameter("owT", [D, D], bf16, isOutput=False)
    qkb = nc.declare_dram_parameter("qkb", [128, 2 * NDT], f32, isOutput=False)
    vbb = nc.declare_dram_parameter("vbb", [128, D], bf16, isOutput=False)
    obb = nc.declare_dram_parameter("obb", [128, D], bf16, isOutput=False)
    out = nc.declare_dram_parameter("out", [P, D], bf16, isOutput=True)

    with tile.TileContext(nc) as tc:
        _emit(tc, nc, tile, mybir, f32, bf16, f8,
              tsh, tsl, llmh, llml, qkw, vwh, owT, qkb, vbb, obb, out)
    nc.compile()
    return nc


def _emit(tc, nc, tile, mybir, f32, bf16, f8,
          tsh, tsl, llmh, llml, qkw, vwh, owT, qkb, vbb, obb, out):
    from contextlib import ExitStack
    from concourse.masks import make_identity

    Exp = mybir.ActivationFunctionType.Exp
    DR = mybir.MatmulPerfMode.DoubleRow
    MUL = mybir.AluOpType.mult
    ADD = mybir.AluOpType.add
    EXP_SCALE = 0.125 / 64.0   # scores carry x8 * x8 from the fp8 stores

    with ExitStack() as ctx:
        persist = ctx.enter_context(tc.tile_pool(name="persist", bufs=1))
        ktpool = ctx.enter_context(tc.tile_pool(name="ktpool", bufs=3))
        qtpool = ctx.enter_context(tc.tile_pool(name="qtpool", bufs=3))
        expool = ctx.enter_context(tc.tile_pool(name="expool", bufs=22))
        rpool = ctx.enter_context(tc.tile_pool(name="rpool", bufs=2))
        opool = ctx.enter_context(tc.tile_pool(name="opool", bufs=5))

        ident = persist.tile([128, 128], bf16, name="ident", tag="ident")
        make_identity(nc, ident)

        # ---- persistent input tiles: ONE tile per tensor, DoubleRow view
        # [128, 8=(dp i), cols].  Element (p, 2dp+i, c) <- dram row
        # dp*256 + i*128 + p, col c.
        def big_tile(name, cols, dt=f8):
            t = persist.tile([128, 8 * cols], dt, name=name, tag=name)
            return t.rearrange("p (g c) -> p g c", g=8)

        ts_h3 = big_tile("ts_h", P)
        ts_l3 = big_tile("ts_l", P)
        llm_h3 = big_tile("llm_h", S)
        llm_l3 = big_tile("llm_l", S)
        qkw3 = big_tile("qkw", 2 * D)
        vw_h3 = big_tile("vw_h", D)
        qkb_sb = persist.tile([128, 2 * NDT], f32, name="qkb_sb", tag="qkb_sb")
        vbb_sb = persist.tile([128, D], bf16, name="vbb_sb", tag="vbb_sb")
        obb_sb = persist.tile([128, D], bf16, name="obb_sb", tag="obb_sb")
        # O weights: one tile, slice [:, d*1024 + jc*512 : ...]
        ow_flat = persist.tile([128, NDT * D], bf16, name="ow_sb", tag="ow_sb")

        def ow_sl(d, jc):
            return ow_flat[:, d * D + jc * 512:d * D + (jc + 1) * 512]

        # combined q/k weight column offsets within the 2048-col inner dim
        def qoff(b):
            return b * 128 if b < 2 else 512 + (b - 2) * 128

        def koff(b):
            return 256 + b * 128 if b < 2 else 1280 + (b - 2) * 128

        # ---- input DMAs: one big transfer each, sync queue only (the Act
        # queue must stay clear of DMA issues or they'd delay the exps;
        # HWDGE is a single shared device anyway).  Consumption order.
        def dma_big(dst3, dram, lo, hi):
            src = dram.ap()[:, lo:hi].rearrange("(g p) c -> p g c", g=8)
            nc.sync.dma_start(out=dst3[:, :, lo:hi], in_=src)

        dma_big(ts_h3, tsh, 0, P)
        nc.sync.dma_start(out=qkb_sb, in_=qkb.ap())
        dma_big(qkw3, qkw, 0, 512)          # q-j01 + k-j01 blocks
        dma_big(llm_h3, llmh, 0, 512)
        dma_big(ts_l3, tsl, 0, P)
        dma_big(llm_h3, llmh, 512, 1024)
        dma_big(vw_h3, vwh, 0, D)
        dma_big(llm_l3, llml, 0, 1024)
        dma_big(qkw3, qkw, 512, 1280)       # q-rest (QT J1 at stage 5)
        dma_big(llm_h3, llmh, 1024, 2048)
        dma_big(llm_l3, llml, 1024, 2048)
        dma_big(qkw3, qkw, 1280, 2048)      # k-rest (kt J1+ from stage 13)
        nc.sync.dma_start(out=vbb_sb, in_=vbb.ap())
        nc.sync.dma_start(out=obb_sb, in_=obb.ap())
        ow4 = ow_flat.rearrange("p (d j) -> p d j", j=D)
        for half in range(2):
            src = owT.ap()[half * 512:(half + 1) * 512, :] \
                .rearrange("(d p) j -> p d j", d=4)
            nc.sync.dma_start(out=ow4[:, half * 4:(half + 1) * 4, :], in_=src)

        # ---- on-chip intermediates ----
        # qt/kt land DIRECTLY in DoubleRow layout: the host permutes the
        # q/k weight columns so output partition q = jtsub*64+u*32+pr of
        # block (J,i) is head-dim j = (2J+jtsub)*128 + u*64 + i*32 + pr.
        # qt_dr[J] is [128, 2, P] (i-major halves), ktdr likewise over S.
        qt_dr = [None] * NDP      # [128, 2*P] f8, 8*q values
        kt_dr = [None] * (NDP + 1)  # [+1: full-precision redo of J0]
        vp_sb = [None] * NST      # [128, H*(DH+1)] bf16
        ctx_nat = persist.tile([128, NPT * D], bf16, name="ctx_nat",
                               tag="ctx_nat")
        cxT = []
        for d in range(NDT):
            cxT.append(persist.tile([128, P], bf16, name=f"cxT{d}",
                                    tag=f"cxT{d}"))
        partial = []
        for T in range(8):
            partial.append(persist.tile([128, 512], bf16, name=f"opart{T}",
                                        tag=f"opart{T}"))

        with tc.tile_pool(name="psS", bufs=2, space="PSUM") as psS, \
             tc.tile_pool(name="psC", bufs=2, space="PSUM") as psC, \
             tc.tile_pool(name="psP", bufs=2, space="PSUM") as psP:

            # ---------------- emission helpers ----------------
            def emit_qt(J, i, hi_only=False, dst_tile=None):
                # qt block (J,i) -> qt_dr[J] half i.  (ts_h+ts_l) x qw_h.
                # hi_only + dst_tile: startup variant into a scratch tile
                # (drops ts_l from the first-exp critical path; only score
                # stages (0,0)/(0,1) of heads 0/1 consume it).
                if dst_tile is None:
                    if qt_dr[J] is None:
                        qt_dr[J] = qtpool.tile([128, 2 * P], f8,
                                               name=f"qt_dr{J}", tag="qt")
                    dst_tile = qt_dr[J]
                b = 2 * J + i
                dst = dst_tile[:, i * P:(i + 1) * P]
                ps = psP.tile([128, P], f32, name=f"ps_q{b}", tag="psP")
                rhs_sets = (ts_h3,) if hi_only else (ts_h3, ts_l3)
                n = len(rhs_sets) * NDP
                g = 0
                for rhs3 in rhs_sets:
                    for dp in range(NDP):
                        nc.tensor.matmul(
                            ps,
                            lhsT=qkw3[:, 2 * dp:2 * dp + 2,
                                      qoff(b):qoff(b) + 128],
                            rhs=rhs3[:, 2 * dp:2 * dp + 2, :],
                            start=(g == 0), stop=(g == n - 1), perf_mode=DR)
                        g += 1
                # psum holds 16*q; store 8*q + 8*qb
                nc.vector.tensor_scalar(dst, ps, 0.5, qkb_sb[:, b:b + 1],
                                        MUL, ADD)

            def emit_kt_sc(slot, J, i, sc, lite=False):
                # kt block (J,i) s-chunk sc -> kt_dr[slot] half i.
                if kt_dr[slot] is None:
                    kt_dr[slot] = ktpool.tile([128, 2 * S], f8,
                                              name=f"kt_dr{slot}", tag="kt")
                b = 2 * J + i
                ps = psP.tile([128, 512], f32, name=f"ps_k{slot}_{b}_{sc}",
                              tag="psP")
                rhs_sets = (llm_h3,) if lite else (llm_h3, llm_l3)
                n = len(rhs_sets) * NDP
                g = 0
                for rhs3 in rhs_sets:
                    for dp in range(NDP):
                        nc.tensor.matmul(
                            ps,
                            lhsT=qkw3[:, 2 * dp:2 * dp + 2,
                                      koff(b):koff(b) + 128],
                            rhs=rhs3[:, 2 * dp:2 * dp + 2,
                                     sc * 512:(sc + 1) * 512],
                            start=(g == 0), stop=(g == n - 1), perf_mode=DR)
                        g += 1
                nc.vector.tensor_scalar(
                    kt_dr[slot][:, i * S + sc * 512:i * S + (sc + 1) * 512],
                    ps, 0.5, qkb_sb[:, NDT + b:NDT + b + 1], MUL, ADD)

            def emit_v(st, jc):
                # V'[s, h*65+x] bf16, heads jc*8..jc*8+8 only.  ctx for
                # pairs 0-3 reads just the jc=0 half, so jc=1 is deferred
                # to pairs 2-4.  psum = 16*v -> *1/16 + vb on evac.
                if vp_sb[st] is None:
                    vp_sb[st] = persist.tile([128, H * (DH + 1)], bf16,
                                             name=f"vp_sb{st}",
                                             tag=f"vp_sb{st}")
                vp3 = vp_sb[st].rearrange("p (h x) -> p h x", x=DH + 1)
                nc.gpsimd.memset(vp3[:, jc * 8:(jc + 1) * 8, DH:DH + 1], 1.0)
                ps = psP.tile([128, 512], f32, name=f"ps_v{st}_{jc}",
                              tag="psP")
                n = 2 * NDP
                i = 0
                for lhs3 in (llm_h3, llm_l3):
                    for dp in range(NDP):
                        nc.tensor.matmul(
                            ps,
                            lhsT=lhs3[:, 2 * dp:2 * dp + 2,
                                      st * 128:(st + 1) * 128],
                            rhs=vw_h3[:, 2 * dp:2 * dp + 2,
                                      jc * 512:(jc + 1) * 512],
                            start=(i == 0), stop=(i == n - 1),
                            perf_mode=DR)
                        i += 1
                nc.vector.scalar_tensor_tensor(
                    vp3[:, jc * 8:(jc + 1) * 8, 0:DH],
                    ps.rearrange("p (h x) -> p h x", x=DH),
                    1.0 / 16.0,
                    vbb_sb[:, jc * 512:(jc + 1) * 512]
                    .rearrange("p (h x) -> p h x", x=DH),
                    MUL, ADD)

            def emit_ctx(p, k, ets):
                # ctx[p-chunk, 0:65] += expT_h(st).T @ V'_h (bf16).
                for i in range(2):
                    st = 2 * k + i
                    for u in range(2):
                        h = 2 * p + u
                        for pc in range(NPT):
                            nc.tensor.matmul(
                                psc[u][:, pc * (DH + 1):
                                       (pc + 1) * (DH + 1)],
                                lhsT=ets[u][:, i * 512 + pc * 128:
                                            i * 512 + (pc + 1) * 128],
                                rhs=vp_sb[st][:, h * (DH + 1):
                                              (h + 1) * (DH + 1)],
                                start=(st == 0 and pc == 0),
                                stop=(st == NST - 1 and pc == NPT - 1))

            def emit_normalize(p, act_split=False):
                rcs = []
                for u in range(2):
                    h = 2 * p + u
                    psc3 = psc[u].rearrange("p (c x) -> p c x", x=DH + 1)
                    rc = rpool.tile([128, NPT], f32, name=f"rc{h}", tag="rc")
                    rc3 = rc.rearrange("p (c x) -> p c x", x=1)
                    nc.vector.reciprocal(rc3, psc3[:, :, DH:DH + 1])
                    rcs.append(rc)
                for pc in range(NPT):
                    for u in range(2):
                        h = 2 * p + u
                        dst = ctx_nat[:, pc * D + h * DH:pc * D + (h + 1) * DH]
                        srcp = psc[u][:, pc * (DH + 1):pc * (DH + 1) + DH]
                        if act_split and u == 1:
                            nc.scalar.mul(dst, srcp, rcs[u][:, pc:pc + 1])
                        else:
                            nc.vector.tensor_scalar_mul(
                                dst, srcp, rcs[u][:, pc:pc + 1])

            def emit_transposes(p, act_split=False):
                for pc in range(NPT):
                    pst = psP.tile([128, 128], bf16, name=f"pst{p}_{pc}",
                                   tag="psP")
                    nc.tensor.transpose(
                        pst,
                        ctx_nat[:, pc * D + p * 128:pc * D + (p + 1) * 128],
                        ident)
                    if act_split and pc % 2 == 1:
                        nc.scalar.copy(cxT[p][:, pc * 128:(pc + 1) * 128],
                                       pst)
                    else:
                        nc.vector.tensor_copy(
                            cxT[p][:, pc * 128:(pc + 1) * 128], pst)

            def emit_opartial(T):
                # out tile T=(pt,jc): bf16 sum d=0..5 plus output bias.
                pt, jc = T // 2, T % 2
                ps = psP.tile([128, 512], f32, name=f"ps_op{T}", tag="psP")
                for d in range(6):
                    nc.tensor.matmul(
                        ps, lhsT=cxT[d][:, pt * 128:(pt + 1) * 128],
                        rhs=ow_sl(d, jc),
                        start=(d == 0), stop=(d == 5))
                nc.vector.tensor_add(partial[T], ps,
                                     obb_sb[:, jc * 512:(jc + 1) * 512])

            # ---------------- prologue ----------------
            emit_qt(0, 0)
            emit_qt(0, 1)
            emit_kt_sc(0, 0, 0, 0, lite=True)
            emit_kt_sc(0, 0, 1, 0, lite=True)

            # ---- filler slot table: global stage -> emissions.  Balances
            # PE work against the 2.08us/stage Act budget; every entry is
            # placed >= ~2 stages before first consumption.
            sched = {}

            def at(g, fn, *args):
                sched.setdefault(g, []).append((fn, args))

            for j in range(12):               # V' heads 0-7: pairs 0-1
                at(j + 2, emit_v, j, 0)
            for j in (12, 13):
                at(14, emit_v, j, 0)
            for j in (14, 15):
                at(15, emit_v, j, 0)
            for j in range(16):               # V' heads 8-15: pairs 2-4
                at(16 + (11 * j) // 8, emit_v, j, 1)
            for s in (1, 2, 3):               # kt J0-lite rest (pair 0)
                at(2 * (s - 1), emit_kt_sc, 0, 0, 0, s, True)
                at(2 * (s - 1), emit_kt_sc, 0, 0, 1, s, True)
            for Jn in (1, 2, 3):              # kt J1..J3, JIT
                for s in range(4):
                    for i in range(2):
                        at(16 * Jn - 3 + 2 * s + i, emit_kt_sc,
                           Jn, Jn, i, s)
            at(5, emit_qt, 1, 0)              # qt J1..J3
            at(6, emit_qt, 1, 1)
            at(25, emit_qt, 2, 0)
            at(27, emit_qt, 2, 1)
            at(41, emit_qt, 3, 0)
            at(43, emit_qt, 3, 1)
            for T in range(6):                # O partials T0-5 in pair 7
                at(58 + T, emit_opartial, T)

            # ---------------- pipelined head pairs ----------------
            # ctx lags ONE PAIR (8 stages): pend queue of stage records.
            psc = None
            pend = []

            for p in range(NPAIR):
                for k in range(8):
                    g = 8 * p + k          # global stage index
                    if psc is None:
                        psc = [psC.tile([128, NPT * (DH + 1)], f32,
                                        name=f"psc{u}", tag="psC")
                               for u in range(2)]
                    pss = [psS.tile([128, 1024], f32,
                                    name=f"ps_s{2*p+u}_{k}", tag="psS")
                           for u in range(2)]
                    J, jsub = p // 2, p % 2
                    k3 = kt_dr[J].rearrange("q (i s) -> q i s", i=2)
                    q3 = qt_dr[J].rearrange("q (i x) -> q i x", i=2)
                    for u in range(2):
                        base = jsub * 64 + u * 32
                        for i in range(2):
                            st = 2 * k + i
                            nc.tensor.matmul(
                                pss[u][:, i * 512:(i + 1) * 512],
                                lhsT=k3[base:base + 32, :,
                                        st * 128:(st + 1) * 128],
                                rhs=q3[base:base + 32, :, :],
                                start=True, stop=True, perf_mode=DR,
                                tile_position=(base, 0))
                    ets = []
                    for u in range(2):
                        et = expool.tile([128, 1024], bf16,
                                         name=f"et{2*p+u}_{k}", tag="et")
                        nc.scalar.activation(et, pss[u], Exp,
                                             bias=0.0, scale=EXP_SCALE)
                        ets.append(et)

                    # ---- PE fillers from the slot table ----
                    for fn, args in sched.pop(g, ()):
                        fn(*args)
                    # transposes(p-2) once normalize(p-2) has run
                    if k == 1 and p >= 2:
                        emit_transposes(p - 2)

                    # ---- lagged ctx: one pair behind ----
                    pend.append((p, k, ets))
                    if len(pend) > 8:
                        cp, ck, cets = pend.pop(0)
                        emit_ctx(cp, ck, cets)
                        if ck == 7:
                            emit_normalize(cp)
                            psc = None

            # ---------------- tail ----------------
            # ctx(7,*) + T6/T7 partials interleaved, then normalize(7),
            # transposes(6,7), final d6/d7 + partial accumulate, out DMA.
            psc = [psC.tile([128, NPT * (DH + 1)], f32,
                            name=f"psc_t{u}", tag="psC") for u in range(2)]
            emit_opartial(6)
            emit_opartial(7)
            for idx in range(8):
                cp, ck, cets = pend.pop(0)
                emit_ctx(cp, ck, cets)
                if ck == 7:
                    emit_normalize(cp, act_split=True)
            emit_transposes(6, act_split=True)
            emit_transposes(7, act_split=True)
            for pc in range(NPT):
                ot = opool.tile([128, 1024], bf16, name=f"ot{pc}", tag="ot",
                                bufs=3)
                for jc in range(2):
                    T = pc * 2 + jc
                    tpool, ttag = ((psS, "psS") if jc == 0 else (psC, "psC"))
                    ps = tpool.tile([128, 512], f32, name=f"ps_o7_{T}",
                                    tag=ttag)
                    for d in (6, 7):
                        nc.tensor.matmul(
                            ps, lhsT=cxT[d][:, pc * 128:(pc + 1) * 128],
                            rhs=ow_sl(d, jc),
                            start=(d == 6), stop=False)
                    nc.tensor.matmul(ps, lhsT=ident, rhs=partial[T],
                                     start=False, stop=True)
                    if jc == 0:
                        nc.scalar.copy(ot[:, 0:512], ps)
                    else:
                        nc.vector.tensor_copy(ot[:, 512:1024], ps)
                    nc.sync.dma_start(
                        out=out.ap()[pc * 128:(pc + 1) * 128,
                                     jc * 512:(jc + 1) * 512],
                        in_=ot[:, jc * 512:(jc + 1) * 512])


def get_nc():
    global _cached_nc
    if _cached_nc is None:
        _cached_nc = _build_nc()
    return _cached_nc


def _split8(x):
    hi = x.astype(_F8)
    lo = (x - hi.astype(np.float32)).astype(_F8)
    return hi, lo


def _dr_perm():
    """Column permutation for q/k weights: new column b*128+q holds
    original head-dim j so the projection lands in DoubleRow layout.
    b = 2J+i, q = jtsub*64 + u*32 + pr -> j = (2J+jtsub)*128+u*64+i*32+pr.
    """
    c = np.arange(D)
    J, i, q = c // 256, (c % 256) // 128, c % 128
    jtsub, u, pr = q // 64, (q % 64) // 32, q % 32
    return (2 * J + jtsub) * 128 + u * 64 + i * 32 + pr


def make_in_maps(ts_features, llm_features, q_w, q_b, k_w, k_b, v_w, v_b,
                 o_w, o_b):
    ts = np.asarray(ts_features, np.float32)
    llm = np.asarray(llm_features, np.float32)
    qwT = np.ascontiguousarray(np.asarray(q_w, np.float32).T)
    kwT = np.ascontiguousarray(np.asarray(k_w, np.float32).T)
    vwT = np.ascontiguousarray(np.asarray(v_w, np.float32).T)
    owT = np.ascontiguousarray(np.asarray(o_w, np.float32).T)
    jmap = _dr_perm()
    qwP = (16.0 * qwT[:, jmap]).astype(_F8)
    kwP = (16.0 * kwT[:, jmap]).astype(_F8)
    # combined layout: [q-j01(256) | k-j01(256) | q-rest(768) | k-rest(768)]
    qkw = np.concatenate(
        [qwP[:, 0:256], kwP[:, 0:256], qwP[:, 256:1024], kwP[:, 256:1024]],
        axis=1)
    shared = {
        "qkw": np.ascontiguousarray(qkw),
        "vwh": np.ascontiguousarray((16.0 * vwT).astype(_F8)),
        "owT": owT.astype(_BF16),
        # biases for the x8-scaled fp8 qt/kt stores, in permuted order
        "qkb": np.ascontiguousarray(np.concatenate(
            [8.0 * np.asarray(q_b, np.float32)[jmap].reshape(NDT, 128).T,
             8.0 * np.asarray(k_b, np.float32)[jmap].reshape(NDT, 128).T],
            axis=1)),
        "vbb": np.ascontiguousarray(
            np.broadcast_to(np.asarray(v_b, np.float32), (128, D))).astype(_BF16),
        "obb": np.ascontiguousarray(
            np.broadcast_to(np.asarray(o_b, np.float32), (128, D))).astype(_BF16),
    }
    in_maps = []
    for b in range(NCORES):
        m = dict(shared)
        tsT = np.ascontiguousarray(ts[b].T)
        llmT = np.ascontiguousarray(llm[b].T)
        m["tsh"], m["tsl"] = _split8(tsT)
        m["llmh"], m["llml"] = _split8(llmT)
        in_maps.append(m)
    return in_maps


def kernel(**inputs):
    from concourse.bass_utils import run_bass_kernel_spmd

    nc = get_nc()
    in_maps = make_in_maps(**inputs)
    res = run_bass_kernel_spmd(nc, in_maps, list(range(NCORES)))
    return np.stack([res.results[i]["out"] for i in range(NCORES)],
                    axis=0).astype(np.float32)
